# revision 7
# baseline (speedup 1.0000x reference)
"""Trainium2 Bass kernel for nn_BasicRNNBlock (vanilla tanh RNN).

Reference semantics (fp32):
    xp = einsum("bti,hi->tbh", x, W_ih) + b_ih + b_hh      # input projection
    h_t = tanh(xp_t + h_{t-1} @ W_hh.T),  h_0 = 0          # T sequential steps
    out[b, t, :] = h_t[b]                                  # [B, T, H]

Shapes: B=64, T=512, I=H=1024.  Sharding: data-parallel over batch across
8 NeuronCores (8 batches/core, weights replicated).  All-fp16 matmul inputs
(measured ~3e-4 rel error per step), fp32 PSUM accumulation.

Per-core device program (SPMD):
  The recurrence state is kept transposed (hT, [128, 64] = [kappa, chunk*8+b])
  so each step's 64 fp16 matmuls (W_hh 128x128 blocks stationary) accumulate
  z^T into PSUM directly in hT-major layout; an identity matmul injects the
  precomputed xp_t (start=True, first in the accumulation group); ACT tanh
  (split in two halves for cross-step pipelining) produces h_t^T which feeds
  the next step and is DMA'd out.  The input projection GEMM (xp) is
  interleaved into the recurrence: one projection matmul per step computes
  the next 64-step slice of xp while the current slice is consumed.
"""
import numpy as np

B, T, I, H = 64, 512, 1024, 1024
N_CORES = 8
BS = B // N_CORES          # 8 batches per core
NCH = H // 128             # 8 chunks of 128 along H
WIN = 64                   # recurrence steps per projection slice (512 cols)
NSLICE = T // WIN          # 8 projection slices


def _build_program(steps=T, interleave=True, split_tanh=True):
    from concourse import bacc, mybir
    import concourse.tile as tile

    f16 = mybir.dt.float16
    f32 = mybir.dt.float32

    nc = bacc.Bacc(None, target_bir_lowering=False)

    wih = nc.declare_dram_parameter("wih", [128, 8192], f16, isOutput=False)
    whh = nc.declare_dram_parameter("whh", [128, 8192], f16, isOutput=False)
    xt = nc.declare_dram_parameter("xt", [128, 8 * 4096], f16, isOutput=False)
    ident = nc.declare_dram_parameter("ident", [128, 128], f16, isOutput=False)
    bias = nc.declare_dram_parameter("bias", [128, 8], f32, isOutput=False)
    y = nc.declare_dram_parameter("y", [steps, 128, 64], f16, isOutput=True)

    n_slices_used = (steps + WIN - 1) // WIN

    with tile.TileContext(nc) as tc:
        with (
            tc.tile_pool(name="const", bufs=1) as const_pool,
            tc.tile_pool(name="xslice", bufs=2) as xslice_pool,
            tc.tile_pool(name="xp", bufs=3) as xp_pool,
            tc.tile_pool(name="hst", bufs=3) as h_pool,
            tc.tile_pool(name="pp", bufs=2, space="PSUM") as proj_psum,
            tc.tile_pool(name="rp", bufs=3, space="PSUM") as rec_psum,
        ):
            wih_sb = const_pool.tile([128, 8192], f16)
            whh_sb = const_pool.tile([128, 8192], f16)
            ident_sb = const_pool.tile([128, 128], f16)
            bias_sb = const_pool.tile([128, 8], f32)
            nc.sync.dma_start(wih_sb[:], wih[:])
            nc.sync.dma_start(whh_sb[:], whh[:])
            nc.sync.dma_start(ident_sb[:], ident[:])
            nc.sync.dma_start(bias_sb[:], bias[:])

            eng_cycle = [nc.sync, nc.scalar, nc.gpsimd]

            def load_xt_slice(s):
                """DMA xt k-chunks for slice s into a fresh [128, 4096] tile."""
                xsl = xslice_pool.tile([128, 8 * 512], f16, name="xsl", tag="xsl")
                for k in range(8):
                    eng_cycle[k % 3].dma_start(
                        xsl[:, k * 512:(k + 1) * 512],
                        xt[:, k * 4096 + s * 512: k * 4096 + (s + 1) * 512],
                    )
                return xsl

            # xp slice tile layout: [kappa, c*512 + local_t*8 + b]
            def proj_block(xsl, xp_tile, c, k, psum_holder):
                if k == 0:
                    psum_holder[0] = proj_psum.tile([128, 512], f32, name="ppsum", tag="ppsum")
                nc.tensor.matmul(
                    psum_holder[0][:],
                    wih_sb[:, k * 1024 + c * 128: k * 1024 + (c + 1) * 128],
                    xsl[:, k * 512:(k + 1) * 512],
                    start=(k == 0), stop=(k == 7),
                )
                if k == 7:
                    nc.scalar.activation(
                        xp_tile[:, c * 512:(c + 1) * 512],
                        psum_holder[0][:],
                        mybir.ActivationFunctionType.Identity,
                        bias=bias_sb[:, c:c + 1],
                    )

            # ---------------- prologue: projection slice 0 ----------------
            xp_tiles = {}
            xsl_tiles = {}
            xsl_tiles[0] = load_xt_slice(0)
            if n_slices_used > 1:
                xsl_tiles[1] = load_xt_slice(1)
            xp_tiles[0] = xp_pool.tile([128, 8 * 512], f16, name="xpt", tag="xpt")
            ph = [None]
            for c in range(NCH):
                for k in range(8):
                    proj_block(xsl_tiles[0], xp_tiles[0], c, k, ph)

            if not interleave:
                for s in range(1, n_slices_used):
                    if s + 1 < n_slices_used and (s + 1) not in xsl_tiles:
                        xsl_tiles[s + 1] = load_xt_slice(s + 1)
                    xp_tiles[s] = xp_pool.tile([128, 8 * 512], f16, name="xpt", tag="xpt")
                    for c in range(NCH):
                        for k in range(8):
                            proj_block(xsl_tiles[s], xp_tiles[s], c, k, ph)

            # ---------------- recurrence ----------------
            h_cur = None
            pph = [None]
            for t in range(steps):
                s = t // WIN
                local = t - s * WIN
                xp3 = xp_tiles[s][:].rearrange("p (c n) -> p c n", c=NCH)

                psum = rec_psum.tile([128, 8, 8], f32)
                nc.tensor.matmul(
                    psum[:], ident_sb[:],
                    xp3[:, :, local * 8:(local + 1) * 8],
                    start=True, stop=(t == 0),
                    skip_group_check=True,
                )

                def wblock(c, k, last):
                    nc.tensor.matmul(
                        psum[:, c, :],
                        whh_sb[:, k * 1024 + c * 128: k * 1024 + (c + 1) * 128],
                        h_cur[:, k * 8:(k + 1) * 8],
                        start=False, stop=last,
                        skip_group_check=True,
                    )

                if t > 0:
                    # c 0-3: k 0-3 needs h half1 only, k 4-7 needs half2
                    for c in range(4):
                        for k in range(8):
                            wblock(c, k, False)
                h_new = h_pool.tile([128, 64], f16)
                if split_tanh:
                    nc.scalar.activation(
                        h_new[:, 0:32],
                        psum[:, 0:4, :].rearrange("p c n -> p (c n)"),
                        mybir.ActivationFunctionType.Tanh,
                    )
                # interleaved projection work for slice s+1
                if interleave:
                    sp = s + 1
                    if sp < n_slices_used:
                        if local == 0:
                            xp_tiles[sp] = xp_pool.tile([128, 8 * 512], f16, name="xpt", tag="xpt")
                        cp, kp = local // 8, local % 8
                        proj_block(xsl_tiles[sp], xp_tiles[sp], cp, kp, pph)
                        if 40 <= local < 48 and sp + 1 < n_slices_used:
                            if local == 40:
                                xsl_tiles[sp + 1] = xslice_pool.tile(
                                    [128, 8 * 512], f16, name="xsl", tag="xsl")
                            k = local - 40
                            eng_cycle[k % 3].dma_start(
                                xsl_tiles[sp + 1][:, k * 512:(k + 1) * 512],
                                xt[:, k * 4096 + (sp + 1) * 512:
                                   k * 4096 + (sp + 2) * 512],
                            )
                if t > 0:
                    for c in range(4, 8):
                        for k in range(8):
                            wblock(c, k, (c == 7 and k == 7))
                if split_tanh:
                    nc.scalar.activation(
                        h_new[:, 32:64],
                        psum[:, 4:8, :].rearrange("p c n -> p (c n)"),
                        mybir.ActivationFunctionType.Tanh,
                    )
                else:
                    nc.scalar.activation(
                        h_new[:], psum[:].rearrange("p c n -> p (c n)"),
                        mybir.ActivationFunctionType.Tanh,
                    )
                nc.sync.dma_start(y[t], h_new[:])
                h_cur = h_new

    nc.compile()
    return nc


_PROGRAM_CACHE = {}
BUILD_KW = {}


def _get_program(steps=T):
    key = (steps, tuple(sorted(BUILD_KW.items())))
    if key not in _PROGRAM_CACHE:
        _PROGRAM_CACHE[key] = _build_program(steps, **BUILD_KW)
    return _PROGRAM_CACHE[key]


def _prep_shared(W_ih, W_hh, b_ih, b_hh):
    # lhsT layout [kappa, k*1024 + j] = W[j, k*128+kappa]
    def to_lhsT(W):
        return np.ascontiguousarray(
            W.T.reshape(8, 128, 1024).transpose(1, 0, 2).reshape(128, 8192)
        )

    wih_np = to_lhsT(np.asarray(W_ih)).astype(np.float16)
    whh_np = to_lhsT(np.asarray(W_hh)).astype(np.float16)
    bias_np = np.ascontiguousarray(
        (np.asarray(b_ih) + np.asarray(b_hh)).astype(np.float32).reshape(8, 128).T
    )
    ident_np = np.eye(128, dtype=np.float16)
    return wih_np, whh_np, bias_np, ident_np


TRACE = False
LAST_RESULT = [None]


def kernel(x, W_ih, W_hh, b_ih, b_hh, _steps=T):
    from concourse.bass_utils import run_bass_kernel_spmd

    x = np.asarray(x)
    steps = _steps
    nc = _get_program(steps)
    wih_np, whh_np, bias_np, ident_np = _prep_shared(W_ih, W_hh, b_ih, b_hh)

    in_maps = []
    for core in range(N_CORES):
        xs = x[core * BS:(core + 1) * BS]          # [8, T, I]
        # xt[kappa, k*4096 + t*8 + b] = x[b, t, k*128+kappa]
        xt_np = np.ascontiguousarray(
            xs.transpose(2, 1, 0)                   # [I, T, B]
            .reshape(8, 128, T * BS)                # [k, kappa, t*8+b]
            .transpose(1, 0, 2)                     # [kappa, k, t*8+b]
            .reshape(128, 8 * 4096)
        ).astype(np.float16)
        in_maps.append({
            "wih": wih_np, "whh": whh_np, "xt": xt_np,
            "ident": ident_np, "bias": bias_np,
        })

    res = run_bass_kernel_spmd(nc, in_maps, list(range(N_CORES)), trace=TRACE)
    LAST_RESULT[0] = res

    out = np.empty((B, T, H), dtype=np.float32)
    for core in range(N_CORES):
        yv = res.results[core]["y"]                 # [steps, 128, 64] fp16
        hb = (
            yv.reshape(steps, 128, 8, 8)
            .transpose(3, 0, 2, 1)                  # [b, t, c, kappa]
            .reshape(BS, steps, H)
            .astype(np.float32)
        )
        out[core * BS:(core + 1) * BS, :steps] = hb
    return out
